# revision 14
# baseline (speedup 1.0000x reference)
"""Bahdanau attention TRN2 kernel.

Shapes (hardcoded): B=32, S=2048, HD=HE=1024, f32.
Sharding: batch B split across 8 NeuronCores (4 batches/core); weights
replicated.

Per-core device program (matmuls in fp32r on the PE, fp32 accumulate):
  q = h @ Wq.T                    (PE, transposed to [d,b] layout)
  per (batch, s-group of 512):
    E tile load  [s,e]            (DMA, natural layout)
    E^T tiles    [e,s]            (PE transpose via identity + copy)
    kT = Wk.T^T @ E^T -> [d,s]    (PE, accumulate over e-tiles)
    t = tanh(kT + q)              (ACT, bias=q per-partition)
    scores = v^T t                (PE, accumulate over d-tiles -> [1,s] @p0)
    e = exp(scores)               (ACT, row at partition 0)
    e^T cols (PE block transpose) * mask (DVE) -> eT
    ctx += eT^T @ E               (PE, accumulate into [1,1024] @p0)
  per batch epilogue at partition 0: masked exp row, sum, reciprocal,
  normalize attn and ctx (DVE) -> DMA out.
"""

import numpy as np

import concourse.bass as bass
import concourse.bacc as bacc
import concourse.tile as tile
import concourse.mybir as mybir
from concourse.bass_utils import run_bass_kernel_spmd

F32 = mybir.dt.float32
F32R = mybir.dt.float32r
AF = mybir.ActivationFunctionType

B, S, HD, HE = 32, 2048, 1024, 1024
NCORES = 8
BPC = B // NCORES       # 4 batches per core
SG = 512                # s-group size
NSG = S // SG           # 4 s-groups
ST = SG // 128          # 4 s-subtiles per group
ET = HE // 128          # 8 e-tiles (contraction)
DT = HD // 128          # 8 d-tiles
NEH = HE // 512         # 2 halves of e for context free dim

_CACHE = {}


def _build_program():
    nc = bacc.Bacc("TRN2", target_bir_lowering=False, debug=False,
                   num_devices=NCORES)

    enc = nc.dram_tensor("enc", [BPC, S, HE], F32R, kind="ExternalInput").ap()
    wk = nc.dram_tensor("wk", [HE, HD], F32R, kind="ExternalInput").ap()    # W_key.T
    wq = nc.dram_tensor("wq", [HD, HD], F32R, kind="ExternalInput").ap()    # W_query.T
    hT = nc.dram_tensor("hT", [HD, BPC], F32R, kind="ExternalInput").ap()
    vt = nc.dram_tensor("vt", [128, DT], F32R, kind="ExternalInput").ap()
    m01 = nc.dram_tensor("m01", [BPC, S], F32, kind="ExternalInput").ap()
    mT = nc.dram_tensor("mT", [128, S // 128, BPC], F32R, kind="ExternalInput").ap()
    ident = nc.dram_tensor("ident", [128, 128], F32R, kind="ExternalInput").ap()
    ctx = nc.dram_tensor("ctx", [BPC, HE], F32, kind="ExternalOutput").ap()
    attn = nc.dram_tensor("attn", [BPC, S], F32, kind="ExternalOutput").ap()

    with tile.TileContext(nc) as tc:
        with (
            tc.tile_pool(name="const", bufs=1) as const,
            tc.tile_pool(name="sb_e", bufs=2) as sb_e,
            tc.tile_pool(name="sb_et", bufs=2) as sb_et,
            tc.tile_pool(name="sb_tanh", bufs=2) as sb_tanh,
            tc.tile_pool(name="sb_row", bufs=1) as sb_row,
            tc.tile_pool(name="sb_erow", bufs=2) as sb_erow,
            tc.tile_pool(name="ps_t", bufs=2, space="PSUM") as ps_t,
            tc.tile_pool(name="ps_k", bufs=2, space="PSUM") as ps_k,
            tc.tile_pool(name="ps_s", bufs=2, space="PSUM") as ps_s,
            tc.tile_pool(name="ps_c", bufs=1, space="PSUM") as ps_c,
        ):
            # ---- constants / small inputs ----
            wk_sb = const.tile([128, ET, HD], F32R)
            nc.sync.dma_start(wk_sb[:], wk.rearrange("(et p) d -> p et d", p=128))
            wq_sb = const.tile([128, ET, HD], F32R)
            nc.sync.dma_start(wq_sb[:], wq.rearrange("(et p) d -> p et d", p=128))
            hT_sb = const.tile([128, ET, BPC], F32R)
            nc.sync.dma_start(hT_sb[:], hT.rearrange("(et p) b -> p et b", p=128))
            v_sb = const.tile([128, DT], F32R)
            nc.sync.dma_start(v_sb[:], vt[:])
            mT_sb = const.tile([128, S // 128, BPC], F32R)
            nc.sync.dma_start(mT_sb[:], mT[:])
            ident_sb = const.tile([128, 128], F32R)
            nc.sync.dma_start(ident_sb[:], ident[:])

            q_sb = const.tile([128, DT, BPC], F32)          # q, [d,(dt,b)]
            eT_sb = const.tile([128, S // 128, BPC], F32R)  # masked exp^T cols

            # ---- q = h @ Wq.T, then transpose to [d, b] ----
            qf_sb = sb_row.tile([128, HD], F32R, tag="qf")
            for eh in range(NEH):
                q_ps = ps_k.tile([128, 512], F32, tag="k_ps")
                for et in range(ET):
                    nc.tensor.matmul(
                        q_ps[0:BPC, :],
                        hT_sb[:, et, :],
                        wq_sb[:, et, eh * 512:(eh + 1) * 512],
                        start=(et == 0), stop=(et == ET - 1),
                    )
                nc.scalar.copy(qf_sb[0:BPC, eh * 512:(eh + 1) * 512],
                               q_ps[0:BPC, :])
            for dt_ in range(DT):
                tq = ps_t.tile([128, 128], F32R, tag="tp")
                nc.tensor.transpose(
                    tq[:], qf_sb[:, dt_ * 128:(dt_ + 1) * 128], ident_sb[:])
                nc.any.tensor_copy(q_sb[:, dt_, :], tq[:, 0:BPC].bitcast(F32))

            # ---- main loop: batch outer, s-group inner ----
            for b in range(BPC):
                erow = sb_erow.tile([128, S], F32R, tag="erow")  # row 0 only
                ctx_ps = ps_c.tile([128, HE], F32, tag="ctx")   # row 0 only
                for g in range(NSG):
                    e_sb = sb_e.tile([128, ST, HE], F32R, tag="e")
                    nc.sync.dma_start(
                        e_sb[:],
                        enc[b, g * SG:(g + 1) * SG, :].rearrange(
                            "(st p) e -> p st e", p=128),
                    )
                    # transpose E -> E^T
                    et_sb = sb_et.tile([128, ET, SG], F32R, tag="et")
                    for st in range(ST):
                        for et in range(ET):
                            tp = ps_t.tile([128, 128], F32R, tag="tp")
                            nc.tensor.transpose(
                                tp[:], e_sb[:, st, et * 128:(et + 1) * 128],
                                ident_sb[:],
                            )
                            nc.any.tensor_copy(
                                et_sb[:, et, st * 128:(st + 1) * 128], tp[:])
                    # kT matmul + tanh + scores
                    s_ps = ps_s.tile([128, SG], F32, tag="s_ps")
                    for dt_ in range(DT):
                        k_ps = ps_k.tile([128, SG], F32, tag="k_ps")
                        for et in range(ET):
                            nc.tensor.matmul(
                                k_ps[:],
                                wk_sb[:, et, dt_ * 128:(dt_ + 1) * 128],
                                et_sb[:, et, :],
                                start=(et == 0), stop=(et == ET - 1),
                            )
                        th = sb_tanh.tile([128, SG], F32R, tag="th")
                        nc.scalar.activation(th[:], k_ps[:], AF.Tanh,
                                             bias=q_sb[:, dt_, b:b + 1])
                        nc.tensor.matmul(
                            s_ps[0:1, :],
                            v_sb[:, dt_:dt_ + 1],
                            th[:],
                            start=(dt_ == 0), stop=(dt_ == DT - 1),
                        )
                    # exp(scores) row (partition 0)
                    nc.scalar.activation(
                        erow[0:1, g * SG:(g + 1) * SG], s_ps[0:1, :], AF.Exp)
                    # transpose exp block; column 0 holds the exp values
                    for st in range(ST):
                        gst = g * ST + st
                        te = ps_t.tile([128, 128], F32R, tag="tp")
                        nc.tensor.transpose(
                            te[:], erow[:, gst * 128:(gst + 1) * 128],
                            ident_sb[:])
                        nc.vector.tensor_mul(
                            eT_sb[:, gst, b:b + 1],
                            te[:, 0:1],
                            mT_sb[:, gst, b:b + 1])
                    # context accumulation (row at partition 0)
                    for st in range(ST):
                        for eh in range(NEH):
                            nc.tensor.matmul(
                                ctx_ps[0:1, eh * 512:(eh + 1) * 512],
                                eT_sb[:, g * ST + st:g * ST + st + 1, b],
                                e_sb[:, st, eh * 512:(eh + 1) * 512],
                                start=(g == 0 and st == 0),
                                stop=(g == NSG - 1 and st == ST - 1),
                            )
                # ---- per-batch epilogue (partition-0 rows) ----
                m01row = sb_row.tile([128, S], F32, tag="m01row")
                nc.sync.dma_start(m01row[0:1, :], m01[b:b + 1, :])
                em = sb_row.tile([128, S], F32, tag="em")
                nc.vector.tensor_mul(em[0:1, :], erow[0:1, :].bitcast(F32),
                                     m01row[0:1, :])
                sums = sb_row.tile([128, 1], F32, tag="sums")
                nc.vector.reduce_sum(sums[0:1, :], em[0:1, :],
                                     axis=mybir.AxisListType.X)
                rec = sb_row.tile([128, 1], F32, tag="rec")
                nc.vector.reciprocal(rec[0:1, :], sums[0:1, :])
                at = sb_row.tile([128, S], F32, tag="at")
                nc.vector.tensor_scalar_mul(at[0:1, :], em[0:1, :], rec[0:1, :])
                ctxrow = sb_row.tile([128, HE], F32, tag="ctxrow")
                nc.vector.tensor_scalar_mul(ctxrow[0:1, :], ctx_ps[0:1, :],
                                            rec[0:1, :])
                nc.sync.dma_start(attn[b:b + 1, :], at[0:1, :])
                nc.sync.dma_start(ctx[b:b + 1, :], ctxrow[0:1, :])

    nc.compile()
    return nc


def _get_program():
    if "nc" not in _CACHE:
        _CACHE["nc"] = _build_program()
    return _CACHE["nc"]


def kernel(decoder_hidden, encoder_outputs, src_mask, W_query, W_key, W_energy,
           _want_trace=False):
    decoder_hidden = np.asarray(decoder_hidden, dtype=np.float32)
    encoder_outputs = np.asarray(encoder_outputs, dtype=np.float32)
    src_mask = np.asarray(src_mask)
    W_query = np.asarray(W_query, dtype=np.float32)
    W_key = np.asarray(W_key, dtype=np.float32)
    W_energy = np.asarray(W_energy, dtype=np.float32)

    nc = _get_program()

    wk = np.ascontiguousarray(W_key.T)     # [HE, HD]
    wq = np.ascontiguousarray(W_query.T)   # [HD, HD]
    vt = np.ascontiguousarray(W_energy[0].reshape(DT, 128).T)  # [128, DT]
    ident = np.eye(128, dtype=np.float32)
    m01_full = src_mask.astype(np.float32)  # [B, S]

    in_maps = []
    for c in range(NCORES):
        bs = slice(c * BPC, (c + 1) * BPC)
        m01c = np.ascontiguousarray(m01_full[bs])          # [BPC, S]
        # mT[p, t, b] = m01[b, t*128 + p]
        mTc = np.ascontiguousarray(
            m01c.T.reshape(S // 128, 128, BPC).transpose(1, 0, 2))
        in_maps.append({
            "enc": np.ascontiguousarray(encoder_outputs[bs]),
            "wk": wk,
            "wq": wq,
            "hT": np.ascontiguousarray(decoder_hidden[bs].T),
            "vt": vt,
            "m01": m01c,
            "mT": mTc,
            "ident": ident,
        })

    res = run_bass_kernel_spmd(nc, in_maps, list(range(NCORES)),
                               trace=_want_trace)
    context = np.concatenate([res.results[c]["ctx"] for c in range(NCORES)], axis=0)
    attn = np.concatenate([res.results[c]["attn"] for c in range(NCORES)], axis=0)
    if _want_trace:
        _CACHE["last_result"] = res
    return context, attn
